# revision 9
# baseline (speedup 1.0000x reference)
"""BERT (12-layer, B=16, S=512, D=768) forward pass on 8 Trainium2 NeuronCores.

Strategy: data-parallel over batch — each of the 8 cores runs the full
12-layer encoder on 2 sequences (1024 tokens). No collectives.

Layouts (per core):
  - Residual stream x: fp32, natural layout [128 partitions (token%128), 8 s-tiles, 768]
  - LN outputs transposed to [d, token] (PE transpose) so QKV/FFN matmuls
    contract over d on partitions; LN gain/bias applied during the
    transposed copy via ScalarE per-partition scale/bias.
  - Attention computed per head with scoresT layout [t, s]: softmax without
    max subtraction (scores bounded ~±2 after 1/8 scaling), probsT consumed
    directly as the stationary operand of attn@V, producing oT [e, s] which
    feeds the output projection without further transposes.
  - Row sums of exp via an appended ones-column on V (one extra matmul col);
    normalization applied to oT with a PE-broadcast reciprocal row.
  - All matmuls bf16 with fp32 PSUM accumulation; residual stream fp32.
"""

import os
import numpy as np
import ml_dtypes

import concourse.bacc as bacc
import concourse.bass as bass
import concourse.mybir as mybir
import concourse.tile as tile
from concourse import bass_utils
from concourse.bass import IndirectOffsetOnAxis
from concourse.masks import make_identity

BF16 = ml_dtypes.bfloat16
F32 = mybir.dt.float32
B16 = mybir.dt.bfloat16

B, S, D, H, DH, L, V = 16, 512, 768, 12, 64, 12, 30522
FF = 4 * D
EPS = 1e-5
NCORES = 8
BL = B // NCORES          # sequences per core
NT = BL * S // 128        # 8 token tiles per core
ST = S // 128             # 4 s-tiles per sequence
DC = D // 128             # 6 d-chunks
FT = FF // 128            # 24 ff tiles
NSL = 384                 # free-dim split of D (768 = 2*384), fits PSUM bank
AFT = DH + 1              # 65: v columns + ones column



def _offsets():
    wb, fb = {}, {}
    o = 0
    for nm, sz in (("wq", L * 128 * DC * D), ("wk", L * 128 * DC * D),
                   ("wv", L * 128 * DC * D), ("wo", L * 128 * DC * D),
                   ("w1p", L * FT * 128 * D), ("w2p", L * FT * 128 * D),
                   ("brows", L * 2 * D), ("lnemb", 2 * D), ("lnf", 2 * D)):
        wb[nm] = o; o += sz
    wb_total = o
    o = 0
    for nm, sz in (("semb", 128 * ST * D), ("lng", L * 128 * 2 * DC),
                   ("lnb", L * 128 * 2 * DC), ("b1t", L * 128 * FT)):
        fb[nm] = o; o += sz
    return wb, wb_total, fb, o


WB_OFF, WB_TOTAL, FB_OFF, FB_TOTAL = _offsets()

Act = mybir.ActivationFunctionType
Alu = mybir.AluOpType


def build_nc(n_layers=L, has_bias=True):
    nc = bacc.Bacc("TRN2", target_bir_lowering=False, debug=False,
                   num_devices=NCORES)
    dt = nc.dram_tensor
    t_idx = dt("idxw", [128, NT], mybir.dt.int32, kind="ExternalInput").ap()
    t_emb = dt("temb", [V, D], B16, kind="ExternalInput").ap()
    t_wb = dt("wb", [WB_TOTAL], B16, kind="ExternalInput").ap()
    t_fb = dt("fb", [FB_TOTAL], F32, kind="ExternalInput").ap()

    def wslice(off, n, p=128):
        return t_wb[off:off + n].rearrange("(p m) -> p m", p=p)

    def fslice(off, n, p=128):
        return t_fb[off:off + n].rearrange("(p m) -> p m", p=p)

    t_semb = fslice(FB_OFF["semb"], 128 * ST * D)
    t_lnemb = wslice(WB_OFF["lnemb"], 2 * D, p=1)
    t_lnf = wslice(WB_OFF["lnf"], 2 * D, p=1)
    t_wq = [wslice(WB_OFF["wq"] + l * 128 * DC * D, 128 * DC * D) for l in range(n_layers)]
    t_wk = [wslice(WB_OFF["wk"] + l * 128 * DC * D, 128 * DC * D) for l in range(n_layers)]
    t_wv = [wslice(WB_OFF["wv"] + l * 128 * DC * D, 128 * DC * D) for l in range(n_layers)]
    t_wo = [wslice(WB_OFF["wo"] + l * 128 * DC * D, 128 * DC * D) for l in range(n_layers)]
    t_w1 = [[wslice(WB_OFF["w1p"] + (l * FT + ft) * 128 * D, 128 * D)
             for ft in range(FT)] for l in range(n_layers)]
    t_w2 = [[wslice(WB_OFF["w2p"] + (l * FT + ft) * 128 * D, 128 * D)
             for ft in range(FT)] for l in range(n_layers)]
    t_lng = [fslice(FB_OFF["lng"] + l * 128 * 2 * DC, 128 * 2 * DC) for l in range(n_layers)]
    t_lnb = [fslice(FB_OFF["lnb"] + l * 128 * 2 * DC, 128 * 2 * DC) for l in range(n_layers)]
    t_b1 = [fslice(FB_OFF["b1t"] + l * 128 * FT, 128 * FT) for l in range(n_layers)]
    t_br = [wslice(WB_OFF["brows"] + l * 2 * D, 2 * D, p=1) for l in range(n_layers)]
    t_out = dt("xo", [NT, 128, D], F32, kind="ExternalOutput").ap()

    with tile.TileContext(nc) as tc:
        _body(tc, n_layers, t_idx, t_emb, t_semb, t_lnemb, t_lnf,
              t_wq, t_wk, t_wv, t_wo, t_w1, t_w2, t_lng, t_lnb, t_b1,
              t_br, t_out, has_bias)
    nc.compile()
    return nc


def _body(tc, n_layers, t_idx, t_emb, t_semb, t_lnemb, t_lnf,
          t_wq, t_wk, t_wv, t_wo, t_w1, t_w2, t_lng, t_lnb, t_b1,
          t_br, t_out, has_bias=True):
    nc = tc.nc
    from contextlib import ExitStack
    ctx = ExitStack()
    with ctx:
        pconst = ctx.enter_context(tc.tile_pool(name="pconst", bufs=1))
        pgb = ctx.enter_context(tc.tile_pool(name="pgb", bufs=2))
        px = ctx.enter_context(tc.tile_pool(name="px", bufs=1))
        pw = ctx.enter_context(tc.tile_pool(name="pw", bufs=1))
        psm = ctx.enter_context(tc.tile_pool(name="psm", bufs=2))
        pw1 = ctx.enter_context(tc.tile_pool(name="pw1", bufs=2))
        pw2 = ctx.enter_context(tc.tile_pool(name="pw2", bufs=3))
        pact = ctx.enter_context(tc.tile_pool(name="pact", bufs=1))
        pprob = ctx.enter_context(tc.tile_pool(name="pprob", bufs=3))
        pyn = ctx.enter_context(tc.tile_pool(name="pyn", bufs=4))
        pstat = ctx.enter_context(tc.tile_pool(name="pstat", bufs=8))
        pbc = ctx.enter_context(tc.tile_pool(name="pbc", bufs=2))
        pfin = ctx.enter_context(tc.tile_pool(name="pfin", bufs=1))
        pps = ctx.enter_context(tc.tile_pool(name="pps", bufs=8, space="PSUM"))

        def ps_tile(shape, dtype=F32):
            return pps.tile(shape, dtype, tag="ps", name="ps")

        # ---- constants ----
        ident = pconst.tile([128, 128], B16, tag="ident", name="ident")
        make_identity(nc, ident[:])
        ones = pconst.tile([1, 128], B16, tag="ones", name="ones")
        nc.vector.memset(ones[:], 1.0)
        epsc = pconst.tile([128, 1], F32, tag="epsc", name="epsc")
        nc.vector.memset(epsc[:], EPS)

        idx_sb = pconst.tile([128, NT], mybir.dt.int32, tag="idx", name="idx")
        nc.sync.dma_start(idx_sb[:], t_idx[:])
        semb_sb = pw.tile([128, ST, D], F32, tag="wq", name="semb")
        nc.sync.dma_start(semb_sb[:], t_semb.rearrange("p (s d) -> p s d", s=ST))
        lnemb_sb = pconst.tile([1, 2 * D], B16, tag="lnemb", name="lnemb")
        nc.sync.dma_start(lnemb_sb[:], t_lnemb)
        lnf_sb = pconst.tile([1, 2 * D], B16, tag="lnf", name="lnf")
        nc.sync.dma_start(lnf_sb[:], t_lnf)

        def bcast_rows(src_row):
            """[1, D] bf16 row -> [128, D] fp32 tile via PE outer product."""
            out = pgb.tile([128, D], F32, tag="gbcast", name="gbcast")
            for sl in range(2):
                ps = ps_tile([128, NSL])
                nc.tensor.matmul(ps[:], ones[:], src_row[:, sl * NSL:(sl + 1) * NSL],
                                 start=True, stop=True)
                nc.scalar.copy(out[:, sl * NSL:(sl + 1) * NSL], ps[:])
            return out

        # ---- embedding ----
        x = px.tile([128, NT, D], F32, tag="x", name="x")
        for i in range(NT):
            tg = pyn.tile([128, D], B16, tag="yn", name="tg")
            nc.gpsimd.indirect_dma_start(
                out=tg[:], out_offset=None, in_=t_emb[:, :],
                in_offset=IndirectOffsetOnAxis(ap=idx_sb[:, i:i + 1], axis=0))
            nc.vector.tensor_add(x[:, i, :], tg[:], semb_sb[:, i % ST, :])

        g_emb = bcast_rows(lnemb_sb[0:1, 0:D])
        b_emb = bcast_rows(lnemb_sb[0:1, D:2 * D])

        def ln_stats(x_slice):
            """Return (mean[128,1], rstd[128,1]) for a [128, D] fp32 slice."""
            st_ = pstat.tile([128, 12], F32, tag="bnst", name="bnst")
            nc.vector.bn_stats(st_[:, 0:6], x_slice[:, 0:NSL])
            nc.vector.bn_stats(st_[:, 6:12], x_slice[:, NSL:2 * NSL])
            ag = pstat.tile([128, 2], F32, tag="bnag", name="bnag")
            nc.vector.bn_aggr(ag[:], st_[:].rearrange("p (c k) -> p c k", k=6))
            sd = pstat.tile([128, 1], F32, tag="sd", name="sd")
            nc.scalar.activation(sd[:], ag[:, 1:2], Act.Sqrt, bias=epsc[:])
            rstd = pstat.tile([128, 1], F32, tag="rstd", name="rstd")
            nc.vector.reciprocal(rstd[:], sd[:])
            return ag, rstd

        for i in range(NT):
            ag, rstd = ln_stats(x[:, i, :])
            nc.vector.tensor_scalar(x[:, i, :], x[:, i, :], ag[:, 0:1], rstd[:],
                                    op0=Alu.subtract, op1=Alu.mult)
            nc.vector.tensor_mul(x[:, i, :], x[:, i, :], g_emb[:])
            nc.vector.tensor_add(x[:, i, :], x[:, i, :], b_emb[:])

        # ---- layers ----
        NP = H // 2              # 6 head pairs
        for l in range(n_layers):
            wq_sb = pw.tile([128, DC, D], B16, tag="wq", name="wq")
            nc.sync.dma_start(wq_sb[:], t_wq[l].rearrange("p (c d) -> p c d", c=DC))
            wk_sb = pw.tile([128, DC, D], B16, tag="wk", name="wk")
            nc.sync.dma_start(wk_sb[:], t_wk[l].rearrange("p (c d) -> p c d", c=DC))
            wv_sb = pw.tile([128, DC, D], B16, tag="wv", name="wv")
            nc.sync.dma_start(wv_sb[:], t_wv[l].rearrange("p (c d) -> p c d", c=DC))
            wo_sb = pw.tile([128, DC, D], B16, tag="wo", name="wo")
            nc.sync.dma_start(wo_sb[:], t_wo[l].rearrange("p (c d) -> p c d", c=DC))
            lng_sb = psm.tile([128, 2 * DC], F32, tag="lng", name="lng")
            nc.sync.dma_start(lng_sb[:], t_lng[l])
            lnb_sb = psm.tile([128, 2 * DC], F32, tag="lnb", name="lnb")
            nc.sync.dma_start(lnb_sb[:], t_lnb[l])
            b1_sb = psm.tile([128, FT], F32, tag="b1", name="b1")
            nc.sync.dma_start(b1_sb[:], t_b1[l])
            if has_bias:
                br_sb = psm.tile([1, 2 * D], B16, tag="br", name="br")
                nc.sync.dma_start(br_sb[:], t_br[l])

            h2Ts = {}

            def ln_transposed(b, half, out_tag, out_bufs=1):
                """LN over x tiles of seq b -> [128, DC, S] bf16 transposed,
                with gain/bias applied during the batched PSUM->SBUF copy.
                half=0 selects ln1 params, half=1 ln2."""
                out_t = pact.tile([128, DC, S], B16, tag=out_tag, name=out_tag,
                                  bufs=out_bufs)
                yns = []
                for st in range(ST):
                    xi = ST * b + st
                    ag, rstd = ln_stats(x[:, xi, :])
                    yn = pyn.tile([128, D], B16, tag="yn", name="yn")
                    nc.vector.tensor_scalar(yn[:], x[:, xi, :], ag[:, 0:1], rstd[:],
                                            op0=Alu.subtract, op1=Alu.mult)
                    yns.append(yn)
                off = half * DC
                for dc in range(DC):
                    ptw = ps_tile([128, S], B16)
                    for st in range(ST):
                        nc.tensor.transpose(ptw[:, st * 128:(st + 1) * 128],
                                            yns[st][:, dc * 128:(dc + 1) * 128],
                                            ident[:])
                    nc.scalar.activation(
                        out_t[:, dc, :], ptw[:], Act.Identity,
                        bias=lnb_sb[:, off + dc:off + dc + 1],
                        scale=lng_sb[:, off + dc:off + dc + 1])
                return out_t

            def attn_phase(b):
                # ---- LN1 -> transposed hT with gain/bias ----
                hT = ln_transposed(b, 0, "hT")

                # ---- QKV projections ----
                qT = pact.tile([128, DC, S], B16, tag="qT", name="qT")
                kT = pact.tile([128, DC, S], B16, tag="kT", name="kT")
                for mt in range(DC):
                    for dst, w in ((qT, wq_sb), (kT, wk_sb)):
                        ps = ps_tile([128, S])
                        for dc in range(DC):
                            nc.tensor.matmul(ps[:], w[:, dc, mt * 128:(mt + 1) * 128],
                                             hT[:, dc, :], start=(dc == 0),
                                             stop=(dc == DC - 1))
                        nc.vector.tensor_copy(dst[:, mt, :], ps[:])

                vaug = pact.tile([128, ST, H * AFT], B16, tag="vaug", name="vaug")
                nc.vector.memset(vaug[:], 1.0)
                for tm in range(ST):
                    for sl in range(2):
                        ps = ps_tile([128, NSL])
                        for dc in range(DC):
                            nc.tensor.matmul(ps[:], hT[:, dc, tm * 128:(tm + 1) * 128],
                                             wv_sb[:, dc, sl * NSL:(sl + 1) * NSL],
                                             start=(dc == 0), stop=(dc == DC - 1))
                        out_ap = vaug[:, tm, sl * 6 * AFT:(sl + 1) * 6 * AFT] \
                            .rearrange("p (h w) -> p h w", w=AFT)[:, :, 0:DH]
                        nc.vector.tensor_copy(out_ap, ps[:])

                # ---- attention, software-pipelined across head PAIRS ----
                # Pair (2m, 2m+1) shares kT/qT chunk m; the two scores MMs use
                # disjoint PE row groups (tile_position po=0/64) and run
                # concurrently on HW.  stage A(m): scoresT + exp for both
                # heads; stage B(m): attn@V (+rowsum) per head; stage C(m):
                # reciprocal + col-tiled PE broadcast pair + normalize.
                oT = pact.tile([128, DC, S], B16, tag="oT", name="oT")
                probs_t = {}
                ops_t = {}
                def stage_a(m):
                    probs = pprob.tile([128, 2, ST, S], B16, tag="probs",
                                       name="probs", bufs=2)
                    probs_t[m] = probs
                    for tt in range(ST):
                        psn = [ps_tile([128, S]) for _ in range(2)]
                        for j in range(2):
                            po = j * 64
                            nc.tensor.matmul(
                                psn[j][:], kT[po:po + 64, m, tt * 128:(tt + 1) * 128],
                                qT[po:po + 64, m, :], start=True, stop=True,
                                tile_position=(po, 0))
                        for j in range(2):
                            nc.scalar.activation(probs[:, j, tt, :], psn[j][:],
                                                 Act.Exp,
                                                 scale=float(1.0 / np.sqrt(DH)))
                def stage_b(m):
                    probs = probs_t[m]
                    opsp = [ps_tile([AFT, S]) for _ in range(2)]
                    ops_t[m] = opsp
                    for j in range(2):
                        h = 2 * m + j
                        for tc_ in range(ST):
                            nc.tensor.matmul(opsp[j][:],
                                             vaug[:, tc_, h * AFT:(h + 1) * AFT],
                                             probs[:, j, tc_, :], start=(tc_ == 0),
                                             stop=(tc_ == ST - 1))
                def stage_c(m):
                    opsp = ops_t.pop(m)
                    probs_t.pop(m)
                    rcb = pstat.tile([1, 2, S], B16, tag="rcb", name="rcb", bufs=2)
                    for j in range(2):
                        rc = pstat.tile([1, S], F32, tag="rc", name="rc", bufs=2)
                        nc.vector.reciprocal(rc[:], opsp[j][DH:AFT, :])
                        nc.vector.tensor_copy(rcb[:, j, :], rc[:])
                    bcp = ps_tile([128, S])
                    nc.tensor.matmul(bcp[0:64, :], ones[0:1, 0:64], rcb[:, 0, :],
                                     start=True, stop=True, tile_position=(0, 0))
                    nc.tensor.matmul(bcp[64:128, :], ones[0:1, 0:64], rcb[:, 1, :],
                                     start=True, stop=True, tile_position=(0, 64))
                    bcs = pbc.tile([128, S], B16, tag="bcs", name="bcs")
                    nc.scalar.copy(bcs[:], bcp[:])
                    for j in range(2):
                        po = j * 64
                        nc.vector.tensor_mul(oT[po:po + 64, m, :],
                                             opsp[j][0:DH, :], bcs[po:po + 64, :])
                for m in range(NP + 2):
                    if m < NP:
                        stage_a(m)
                    if 1 <= m < NP + 1:
                        stage_b(m - 1)
                    if m >= 2:
                        stage_c(m - 2)

                # ---- output projection + residual ----
                for st in range(ST):
                    xi = ST * b + st
                    for sl in range(2):
                        ps = ps_tile([128, NSL])
                        for ec in range(DC):
                            nc.tensor.matmul(ps[:], oT[:, ec, st * 128:(st + 1) * 128],
                                             wo_sb[:, ec, sl * NSL:(sl + 1) * NSL],
                                             start=(ec == 0),
                                             stop=(ec == DC - 1 and not has_bias))
                        if has_bias:
                            nc.tensor.matmul(ps[:], ones[:],
                                             br_sb[0:1, sl * NSL:(sl + 1) * NSL],
                                             start=False, stop=True)
                        nc.vector.tensor_add(x[:, xi, sl * NSL:(sl + 1) * NSL],
                                             x[:, xi, sl * NSL:(sl + 1) * NSL], ps[:])

                # ---- LN2 -> h2T ----
                h2Ts[b] = ln_transposed(b, 1, "h2T", out_bufs=2)

            def ffn_phase():
                # ---- FFN up, W1 chunks shared across both seqs ----
                ffTs = [pact.tile([128, FT, S], B16, tag="ffT", name="ffT",
                                  bufs=2) for _ in range(BL)]
                for ft in range(FT):
                    w1c = pw1.tile([128, D], B16, tag="w1c", name="w1c")
                    nc.sync.dma_start(w1c[:], t_w1[l][ft])
                    for b in range(BL):
                        ps = ps_tile([128, S])
                        for dc in range(DC):
                            nc.tensor.matmul(ps[:], w1c[:, dc * 128:(dc + 1) * 128],
                                             h2Ts[b][:, dc, :], start=(dc == 0),
                                             stop=(dc == DC - 1))
                        nc.scalar.activation(ffTs[b][:, ft, :], ps[:], Act.Relu,
                                             bias=b1_sb[:, ft:ft + 1])
                h2Ts.clear()

                # ---- FFN down + residual: two half-column passes over W2 ----
                for sl in range(2):
                    pss = [ps_tile([128, NSL]) for _ in range(BL * ST)]
                    for fc in range(FT):
                        w2c = pw2.tile([128, NSL], B16, tag="w2c", name="w2c")
                        nc.sync.dma_start(w2c[:],
                                          t_w2[l][fc][:, sl * NSL:(sl + 1) * NSL])
                        for b in range(BL):
                            for st in range(ST):
                                nc.tensor.matmul(
                                    pss[b * ST + st][:],
                                    ffTs[b][:, fc, st * 128:(st + 1) * 128],
                                    w2c[:], start=(fc == 0),
                                    stop=(fc == FT - 1 and not has_bias))
                    for b in range(BL):
                        for st in range(ST):
                            xi = ST * b + st
                            k = b * ST + st
                            if has_bias:
                                nc.tensor.matmul(
                                    pss[k][:], ones[:],
                                    br_sb[0:1, D + sl * NSL:D + (sl + 1) * NSL],
                                    start=False, stop=True)
                            nc.vector.tensor_add(x[:, xi, sl * NSL:(sl + 1) * NSL],
                                                 x[:, xi, sl * NSL:(sl + 1) * NSL],
                                                 pss[k][:])

            for b in range(BL):
                attn_phase(b)
            ffn_phase()
        # ---- final LN + store ----
        g_f = bcast_rows(lnf_sb[0:1, 0:D])
        b_f = bcast_rows(lnf_sb[0:1, D:2 * D])
        for i in range(NT):
            ag, rstd = ln_stats(x[:, i, :])
            fo = pfin.tile([128, D], F32, tag="fo", name="fo")
            nc.vector.tensor_scalar(fo[:], x[:, i, :], ag[:, 0:1], rstd[:],
                                    op0=Alu.subtract, op1=Alu.mult)
            nc.vector.tensor_mul(fo[:], fo[:], g_f[:])
            nc.vector.tensor_add(fo[:], fo[:], b_f[:])
            nc.sync.dma_start(t_out[i], fo[:])


def prepare_inputs(inputs, n_layers=L):
    """Host-side shard/pack. Returns list of 8 per-core input maps."""
    f32 = np.float32
    idx = np.asarray(inputs["idx"]).astype(np.int32)           # [B, S]
    tok = np.ascontiguousarray(np.asarray(inputs["tok_emb"], dtype=f32).astype(BF16))
    seg_emb = np.asarray(inputs["seg_emb"], dtype=f32)
    pos = np.asarray(inputs["pos_emb"], dtype=f32)
    seg_pat = np.zeros(S, np.int64); seg_pat[S // 2 + 1:] = 1
    static = (pos[:S] + seg_emb[seg_pat]).astype(f32)          # [S, D]
    semb = np.ascontiguousarray(static.reshape(ST, 128, D).transpose(1, 0, 2))

    def rows2(g, b):
        return np.concatenate([np.asarray(g), np.asarray(b)])[None].astype(f32).astype(BF16)

    lnemb = rows2(inputs["ln_emb_g"], inputs["ln_emb_b"])
    lnf = rows2(inputs["lnf_g"], inputs["lnf_b"])

    sl = slice(0, n_layers)
    Wq = np.asarray(inputs["Wq"], dtype=f32)[sl]
    Wk = np.asarray(inputs["Wk"], dtype=f32)[sl]
    Wv = np.asarray(inputs["Wv"], dtype=f32)[sl]
    Wo = np.asarray(inputs["Wo"], dtype=f32)[sl]
    W1 = np.asarray(inputs["W1"], dtype=f32)[sl]
    W2 = np.asarray(inputs["W2"], dtype=f32)[sl]
    nl = n_layers

    def packw(w):  # [nl, D(d), D(m)] -> [nl, 128(p), DC(dc), D(m)] bf16
        return np.ascontiguousarray(
            w.reshape(nl, DC, 128, D).transpose(0, 2, 1, 3)).astype(BF16)

    wq = packw(Wq.transpose(0, 2, 1, 3).reshape(nl, D, D))
    wk = packw(Wk.transpose(0, 2, 1, 3).reshape(nl, D, D))
    wv = packw(Wv.transpose(0, 2, 1, 3).reshape(nl, D, D))
    wo = packw(Wo)
    w1p = np.ascontiguousarray(
        W1.reshape(nl, DC, 128, FT, 128).transpose(0, 3, 2, 1, 4)
        .reshape(nl, FT, 128, D)).astype(BF16)
    w2p = np.ascontiguousarray(W2.reshape(nl, FT, 128, D)).astype(BF16)

    lng = np.ascontiguousarray(np.concatenate([
        np.asarray(inputs["ln1_g"], dtype=f32)[sl].reshape(nl, DC, 128),
        np.asarray(inputs["ln2_g"], dtype=f32)[sl].reshape(nl, DC, 128)],
        axis=1).transpose(0, 2, 1))
    lnb = np.ascontiguousarray(np.concatenate([
        np.asarray(inputs["ln1_b"], dtype=f32)[sl].reshape(nl, DC, 128),
        np.asarray(inputs["ln2_b"], dtype=f32)[sl].reshape(nl, DC, 128)],
        axis=1).transpose(0, 2, 1))
    b1t = np.ascontiguousarray(
        np.asarray(inputs["b1"], dtype=f32)[sl].reshape(nl, FT, 128)
        .transpose(0, 2, 1))
    brows = np.concatenate([np.asarray(inputs["bo"], dtype=f32)[sl],
                            np.asarray(inputs["b2"], dtype=f32)[sl]],
                           axis=1)[:, None, :].astype(BF16)

    wb = np.empty(WB_TOTAL, BF16)
    def put_w(nm, arr):
        a = np.ascontiguousarray(arr).reshape(-1)
        wb[WB_OFF[nm]:WB_OFF[nm] + a.size] = a
    put_w("wq", wq); put_w("wk", wk); put_w("wv", wv); put_w("wo", wo)
    put_w("w1p", w1p); put_w("w2p", w2p); put_w("brows", brows)
    put_w("lnemb", lnemb); put_w("lnf", lnf)
    fb = np.empty(FB_TOTAL, np.float32)
    def put_f(nm, arr):
        a = np.ascontiguousarray(arr).reshape(-1)
        fb[FB_OFF[nm]:FB_OFF[nm] + a.size] = a
    put_f("semb", semb); put_f("lng", lng); put_f("lnb", lnb); put_f("b1t", b1t)

    shared = dict(temb=tok, wb=wb, fb=fb)
    in_maps = []
    for c in range(NCORES):
        flat = idx[BL * c:BL * (c + 1)].reshape(-1)            # [1024]
        idxw = np.ascontiguousarray(flat.reshape(NT, 128).T)   # [128, NT]
        in_maps.append(dict(idxw=idxw, **shared))
    return in_maps


def assemble_output(results):
    out = np.empty((B, S, D), np.float32)
    for c in range(NCORES):
        xo = results[c]["xo"]                                   # [NT, 128, D]
        for j in range(NT):
            out[BL * c + j // ST, (j % ST) * 128:(j % ST + 1) * 128, :] = xo[j]
    return out


_NC_CACHE = {}


def inputs_have_bias(inputs):
    return bool(np.any(np.asarray(inputs["bo"])) or np.any(np.asarray(inputs["b2"])))


def get_nc(n_layers=L, has_bias=True):
    key = (n_layers, has_bias)
    if key not in _NC_CACHE:
        _NC_CACHE[key] = build_nc(n_layers, has_bias)
    return _NC_CACHE[key]


def kernel(**inputs):
    hb = inputs_have_bias(inputs)
    nc = get_nc(L, hb)
    in_maps = prepare_inputs(inputs, L)
    res = bass_utils.run_bass_kernel_spmd(nc, in_maps, core_ids=list(range(NCORES)))
    return assemble_output(res.results)

